# revision 1
# baseline (speedup 1.0000x reference)
"""MixedQLinear (QUIK-style int4+fp16 outlier linear) on 8 TRN2 NeuronCores.

Sharding: token-parallel. x [4,2048,4096] -> 8192 tokens, 1024 per core;
weights replicated. Each core quantizes its tokens, runs the int4 GEMM
(exact in fp16 PE arithmetic: products of small ints accumulate exactly in
fp32 PSUM) plus the fp-outlier GEMM, dequantizes, and writes its [1024,4096]
slice of the output. Host concatenates.

Key algebra: with r = clip(round((x-mn)/scale),0,15) = q+8,
  out = (sum_k r*Wint) * scale * ws  +  mn*reduced_w  +  fp_x@Wfp^T + bias
(the -8 shift folds exactly against zero*reduced_w). mn*reduced_w and bias
ride as two extra contraction rows of the fp-outlier matmul.
"""

import numpy as np
import concourse.bass as bass
import concourse.tile as tile
import concourse.mybir as mybir
from concourse.bass_utils import run_bass_kernel_spmd
from bass_rust import ScopedClock, SyncInfo
from concourse.alu_op_type import AluOpType

# ---------------------------------------------------------------------------
# Workaround: this toolchain's walrus accepts at most one sync-wait on a
# TPB_CTRL (Drain) instruction; Tile's tail drain attaches one wait per
# active DMA queue. Split it into a chain of single-wait drains.
def _drain_and_barrier(self, tick_clock, wait_clock):
    drain_inst = self.nc.sync.drain()
    wait_clock.add_sem_waits(
        drain_inst.ins, ScopedClock({None: tick_clock.global_clock})
    )
    si = drain_inst.ins.sync_info
    ow = list(si.on_wait) if si is not None else []
    if len(ow) > 1:
        si.on_wait = [ow[0]]
        for w in ow[1:]:
            d2 = self.nc.sync.drain()
            d2.ins.sync_info = SyncInfo(on_wait=[w], on_update=[])
    self.nc.all_engine_barrier()
    assert self.sems is not None
    popped = self.nc._tile_sem_poison_stack.pop()
    assert popped is self._sem_poison
    self.nc.clear_and_free_semaphores(list(self.sems.allocated().values()))
    self.nc.all_engine_barrier()


tile.TileContext._drain_and_barrier = _drain_and_barrier


def _split_multiwait_instructions(nc):
    """Walrus here allows only one sync-wait per instruction: hoist extra
    waits onto same-engine NOPs inserted immediately before."""
    ctr = 0
    for fn in nc.m.functions:
        for bb in fn.blocks:
            insts = bb.instructions
            out = []
            changed = False
            for ins in insts:
                si = getattr(ins, "sync_info", None)
                ow = list(si.on_wait) if si is not None else []
                if len(ow) > 1:
                    changed = True
                    for w in ow[:-1]:
                        ctr += 1
                        out.append(
                            mybir.InstNoOp(
                                name=f"mwsplit-{ctr}",
                                sync_info=SyncInfo(on_wait=[w], on_update=[]),
                                engine=ins.engine,
                                bass_nofuse=True,
                            )
                        )
                    si.on_wait = [ow[-1]]
                out.append(ins)
            if changed:
                bb.instructions = out
# ---------------------------------------------------------------------------

N_CORES = 8
B, S, IN, OUT, FP = 4, 2048, 4096, 4096, 256
NT = (B * S) // N_CORES          # 1024 tokens per core
P = 128
KC = IN // P                     # 32 feature chunks
NOUT = 8                         # out-feature chunks
NSZ = OUT // NOUT                # 512
HT = 2                           # token halves (512) for quantize layout
HSZ = NT // HT                   # 512
TOKT = NT // P                   # 8 token tiles of 128
BIG = 30000.0

f16 = mybir.dt.float16
f32 = mybir.dt.float32
i8 = mybir.dt.int8

_prog_cache = {}


def _build_program():
    nc = bass.Bass()
    xs = nc.declare_dram_parameter("xs", [NT, IN], f16, isOutput=False)
    fpx = nc.declare_dram_parameter("fpx", [NT, FP], f16, isOutput=False)
    wint = nc.declare_dram_parameter("wint", [IN, OUT], f16, isOutput=False)
    wfp0 = nc.declare_dram_parameter("wfp0", [P, OUT], f16, isOutput=False)
    wfp1 = nc.declare_dram_parameter("wfp1", [P, OUT], f16, isOutput=False)
    wfp2 = nc.declare_dram_parameter("wfp2", [2, OUT], f16, isOutput=False)
    wsrow = nc.declare_dram_parameter("wsrow", [OUT], f16, isOutput=False)
    bmrow = nc.declare_dram_parameter("bmrow", [IN], f16, isOutput=False)
    rwrow = nc.declare_dram_parameter("rwrow", [OUT], f16, isOutput=False)
    out_d = nc.declare_dram_parameter("out", [NT, OUT], f16, isOutput=True)

    def bcast(ap, parts=P):
        # DRAM row -> all partitions: stride-0 partition dim, SWDGE DMA
        return bass.AP(
            tensor=ap.tensor, offset=ap.offset, ap=[[0, parts]] + list(ap.ap)
        )

    import contextlib
    with tile.TileContext(nc) as tc:
        with (
            tc.tile_pool(name="const", bufs=1) as cpool,
            tc.tile_pool(name="rt", bufs=1) as rtpool,
            tc.tile_pool(name="stat1", bufs=8) as s1pool,
            tc.tile_pool(name="psum", bufs=3, space="PSUM") as ppool,
            tc.tile_pool(name="dram", bufs=1, space="DRAM") as dpool,
        ):
            # ---- resident constants -------------------------------------
            bmB = cpool.tile([P, IN], f16, tag="bmB")
            nc.gpsimd.dma_start(bmB[:], bcast(bmrow[:]))
            wsB = cpool.tile([P, OUT], f16, tag="wsB")
            nc.gpsimd.dma_start(wsB[:], bcast(wsrow[:]))
            rwB = cpool.tile([P, OUT], f16, tag="rwB")
            nc.gpsimd.dma_start(rwB[:], bcast(rwrow[:]))
            wfp0_s = cpool.tile([P, OUT], f16, tag="wfp0")
            nc.sync.dma_start(wfp0_s[:], wfp0[:])
            wfp1_s = cpool.tile([P, OUT], f16, tag="wfp1")
            nc.sync.dma_start(wfp1_s[:], wfp1[:])
            wfp2_s = cpool.tile([2, OUT], f16, tag="wfp2")
            nc.sync.dma_start(wfp2_s[:], wfp2[:])

            # fp-outlier activations, transposed [feat, tok]
            fpt0 = cpool.tile([P, NT], f16, tag="fpt0")
            fpt1 = cpool.tile([P, NT], f16, tag="fpt1")
            fpt2 = cpool.tile([2, NT], f16, tag="fpt2")
            for h in range(HT):
                hs = slice(h * HSZ, (h + 1) * HSZ)
                nc.sync.dma_start_transpose(fpt0[:, hs], fpx[hs, 0:P])
                nc.sync.dma_start_transpose(fpt1[:, hs], fpx[hs, P:FP])
            nc.vector.memset(fpt2[:], 1.0)  # row1: ones (bias row)
            nc.vector.memset(fpt2[0:1, :], 0.0)  # row0 unused (zero*rw done on DVE)

            # DRAM staging for per-token stats rows
            mn32_d = dpool.tile([NT], f32, tag="mn32")
            inv32_d = dpool.tile([NT], f32, tag="inv32")

            # ---- phase S: per-token masked min/max -> scale/inv ---------
            scl = []  # per-tok-tile [128,1] f32 scale, kept for dequant
            zro = []  # per-tok-tile [128,1] f32 zero-point
            sctx = contextlib.ExitStack()
            spool = sctx.enter_context(tc.tile_pool(name="stat", bufs=2))
            for t in range(TOKT):
                ts_ = slice(t * P, (t + 1) * P)
                xtile = spool.tile([P, IN], f16, tag="xtile")
                nc.sync.dma_start(xtile[:], xs[ts_, :])
                scratch = spool.tile([P, IN], f16, tag="scratch")
                mn_t = s1pool.tile([P, 1], f32, tag=f"mn{t}")
                mx_t = s1pool.tile([P, 1], f32, tag=f"mx{t}")
                nc.vector.tensor_tensor(scratch[:], xtile[:], bmB[:], AluOpType.add)
                nc.vector.tensor_reduce(
                    mn_t[:], scratch[:], mybir.AxisListType.X, AluOpType.min
                )
                nc.vector.tensor_tensor(
                    scratch[:], xtile[:], bmB[:], AluOpType.subtract
                )
                nc.vector.tensor_reduce(
                    mx_t[:], scratch[:], mybir.AxisListType.X, AluOpType.max
                )
                sc_t = s1pool.tile([P, 1], f32, tag=f"sc{t}")
                nc.vector.tensor_tensor(sc_t[:], mx_t[:], mn_t[:], AluOpType.subtract)
                nc.vector.tensor_scalar(
                    sc_t[:], sc_t[:], 1.0 / 15.0, 1e-8,
                    AluOpType.mult, AluOpType.max,
                )
                inv_t = s1pool.tile([P, 1], f32, tag=f"inv{t}")
                nc.vector.reciprocal(inv_t[:], sc_t[:])
                # Newton step: inv *= (2 - scale*inv)  -> f32-accurate 1/scale
                nwt = s1pool.tile([P, 1], f32, tag=f"nw{t}")
                nc.vector.tensor_tensor(nwt[:], sc_t[:], inv_t[:], AluOpType.mult)
                nc.vector.tensor_scalar(
                    nwt[:], nwt[:], -1.0, 2.0, AluOpType.mult, AluOpType.add
                )
                nc.vector.tensor_tensor(inv_t[:], inv_t[:], nwt[:], AluOpType.mult)
                zr_t = s1pool.tile([P, 1], f32, name=f"zr{t}", tag=f"zr{t}")
                nc.vector.tensor_scalar(
                    zr_t[:], sc_t[:], 8.0, mn_t[:, 0:1],
                    AluOpType.mult, AluOpType.add,
                )
                nc.sync.dma_start(mn32_d[ts_], mn_t[:])
                nc.sync.dma_start(inv32_d[ts_], inv_t[:])
                scl.append(sc_t)
                zro.append(zr_t)
            sctx.close()

            # ---- phase Q: quantize into rt[k][feat, tok] ----------------
            qctx = contextlib.ExitStack()
            qpool = qctx.enter_context(tc.tile_pool(name="qw", bufs=3))
            bcpool = qctx.enter_context(tc.tile_pool(name="bc", bufs=2))
            rt = [
                [rtpool.tile([P, HSZ], f16, name=f"rt{k}_{h}", tag=f"rt{k}_{h}") for h in range(HT)]
                for k in range(KC)
            ]
            for h in range(HT):
                hs = slice(h * HSZ, (h + 1) * HSZ)
                mnB = bcpool.tile([P, HSZ], f32, tag="mnB")
                nc.gpsimd.dma_start(mnB[:], bcast(mn32_d[hs]))
                invB = bcpool.tile([P, HSZ], f32, tag="invB")
                nc.gpsimd.dma_start(invB[:], bcast(inv32_d[hs]))
                for k in range(KC):
                    xt = qpool.tile([P, HSZ], f16, tag="xt")
                    nc.sync.dma_start_transpose(
                        xt[:], xs[hs, k * P : (k + 1) * P]
                    )
                    q = qpool.tile([P, HSZ], f32, tag="q")
                    nc.vector.tensor_tensor(q[:], xt[:], mnB[:], AluOpType.subtract)
                    nc.vector.tensor_tensor(q[:], q[:], invB[:], AluOpType.mult)
                    r8 = qpool.tile([P, HSZ], i8, tag="r8")
                    nc.scalar.copy(r8[:], q[:])       # f32->i8 cast: round-half-even
                    nc.scalar.activation(
                        rt[k][h][:], r8[:],
                        mybir.ActivationFunctionType.Copy, bias=-8.0,
                    )  # i8->f16 with the -8 zero shift, exact

            # ---- phase M: matmuls + dequant -----------------------------
            qctx.close()
            mctx = contextlib.ExitStack()
            wpool = mctx.enter_context(tc.tile_pool(name="wp", bufs=2))
            dqpool = mctx.enter_context(tc.tile_pool(name="dq", bufs=2))
            KHALF = KC // 2
            for n in range(NOUT):
                ns = slice(n * NSZ, (n + 1) * NSZ)
                wn0 = wpool.tile([P, KHALF, NSZ], f16, name="wn0", tag="wn0")
                nc.sync.dma_start(
                    wn0[:],
                    wint[0 : KHALF * P, ns].rearrange("(k p) j -> p k j", p=P),
                )
                wn1 = wpool.tile([P, KHALF, NSZ], f16, name="wn1", tag="wn1")
                nc.sync.dma_start(
                    wn1[:],
                    wint[KHALF * P : IN, ns].rearrange("(k p) j -> p k j", p=P),
                )
                for t in range(TOKT):
                    h = t // (TOKT // HT)
                    tsl = slice((t % (TOKT // HT)) * P, (t % (TOKT // HT)) * P + P)
                    ts_ = slice(t * P, (t + 1) * P)
                    psum_i = ppool.tile([P, NSZ], f32, tag="pi")
                    for k in range(KC):
                        wk = wn0[:, k, :] if k < KHALF else wn1[:, k - KHALF, :]
                        nc.tensor.matmul(
                            psum_i[:], rt[k][h][:, tsl], wk,
                            start=(k == 0), stop=(k == KC - 1),
                        )
                    psum_f = ppool.tile([P, NSZ], f32, tag="pf")
                    nc.tensor.matmul(
                        psum_f[:], fpt0[:, ts_], wfp0_s[:, ns], start=True, stop=False
                    )
                    nc.tensor.matmul(
                        psum_f[:], fpt1[:, ts_], wfp1_s[:, ns], start=False, stop=False
                    )
                    nc.tensor.matmul(
                        psum_f[:], fpt2[:, ts_], wfp2_s[:, ns], start=False, stop=True
                    )
                    td = dqpool.tile([P, NSZ], f32, tag="td")
                    nc.scalar.activation(
                        td[:], psum_i[:], mybir.ActivationFunctionType.Copy,
                        scale=scl[t][:, 0:1],
                    )
                    nc.vector.tensor_tensor(td[:], td[:], wsB[:, ns], AluOpType.mult)
                    zc = dqpool.tile([P, NSZ], f32, tag="zc")
                    nc.vector.tensor_scalar(
                        zc[:], rwB[:, ns], zro[t][:, 0:1], None, AluOpType.mult
                    )
                    nc.vector.tensor_tensor(td[:], td[:], zc[:], AluOpType.add)
                    outt = dqpool.tile([P, NSZ], f16, tag="outt")
                    nc.vector.tensor_tensor(outt[:], td[:], psum_f[:], AluOpType.add)
                    nc.sync.dma_start(out_d[ts_, ns], outt[:])
            mctx.close()
    _split_multiwait_instructions(nc)
    return nc


def _get_program():
    if "nc" not in _prog_cache:
        _prog_cache["nc"] = _build_program()
    return _prog_cache["nc"]


def kernel(x, int_weight, fp_weight, bias, weights_scales, reduced_w,
           int_indices, fp_indices):
    x2 = np.asarray(x, dtype=np.float16).reshape(-1, IN)
    ii = np.asarray(int_indices).astype(np.int64)
    fi = np.asarray(fp_indices).astype(np.int64)

    wint_emb = np.zeros((IN, OUT), dtype=np.float16)
    wint_emb[ii, :] = np.asarray(int_weight).astype(np.float16).T
    wfp_all = np.ascontiguousarray(np.asarray(fp_weight, dtype=np.float16).T)
    wfp2 = np.stack([
        np.asarray(reduced_w, dtype=np.float16).reshape(-1),
        np.asarray(bias, dtype=np.float16).reshape(-1),
    ])
    wsrow = np.ascontiguousarray(
        np.asarray(weights_scales, dtype=np.float16).reshape(-1)
    )
    bmrow = np.zeros(IN, dtype=np.float16)
    bmrow[fi] = BIG

    nc = _get_program()
    in_maps = []
    for c in range(N_CORES):
        xsh = x2[c * NT : (c + 1) * NT]
        in_maps.append({
            "xs": np.ascontiguousarray(xsh),
            "fpx": np.ascontiguousarray(xsh[:, fi]),
            "wint": wint_emb,
            "wfp0": np.ascontiguousarray(wfp_all[0:P]),
            "wfp1": np.ascontiguousarray(wfp_all[P:FP]),
            "wfp2": wfp2,
            "wsrow": wsrow,
            "bmrow": bmrow,
            "rwrow": np.ascontiguousarray(
                np.asarray(reduced_w, dtype=np.float16).reshape(-1)
            ),
        })
    res = run_bass_kernel_spmd(nc, in_maps, list(range(N_CORES)))
    out = np.concatenate(
        [res.results[c]["out"] for c in range(N_CORES)], axis=0
    )
    return out.reshape(B, S, OUT).astype(np.float16)



# revision 5
# speedup vs baseline: 1.3680x; 1.3680x over previous
"""MixedQLinear (QUIK-style int4+fp16 outlier linear) on 8 TRN2 NeuronCores.

Sharding: token-parallel. x [4,2048,4096] -> 8192 tokens, 1024 per core;
weights replicated. Each core quantizes its tokens, runs the int4 GEMM in
fp8e4 DoubleRow mode (exact: products of ints in [-8,7] are exact in the
e6m3 PE datapath, accumulated in fp32 PSUM), and writes its [1024,4096]
output slice. Host concatenates.

Key algebra: with r = clip(round((x-mn)/scale),0,15) - 8,
  out = (sum_k r*Wint) * scale * ws  +  zero*reduced_w  +  fp_x@Wfp^T + bias
     = [ sum_k r*Wint + fp_x/scale @ (Wfp/ws)^T + (8+mn/scale)*(rw/ws)
         + (1/scale)*(bias/ws) ] * scale * ws
so the zero-point correction and bias ride as extra contraction rows of the
fp-outlier matmul (scaled by 1/scale per token), and dequant is a single
scaled copy plus one multiply by ws.

The int contraction uses the compact 3840 int-feature order (host gathers
x[:, int_indices] and pre-transposes), so the GEMM does 15 k-chunks of 256
(fp8 DoubleRow pairs) instead of 32 chunks of 128.
"""

import numpy as np
import ml_dtypes
import concourse.bass as bass
import concourse.tile as tile
import concourse.mybir as mybir
from concourse.bass_utils import run_bass_kernel_spmd
from bass_rust import ScopedClock, SyncInfo
from concourse.alu_op_type import AluOpType

# ---------------------------------------------------------------------------
# Workaround: this toolchain's walrus accepts at most one sync-wait on a
# TPB_CTRL (Drain) instruction; Tile's tail drain attaches one wait per
# active DMA queue. Split it into a chain of single-wait drains.
def _drain_and_barrier(self, tick_clock, wait_clock):
    drain_inst = self.nc.sync.drain()
    wait_clock.add_sem_waits(
        drain_inst.ins, ScopedClock({None: tick_clock.global_clock})
    )
    si = drain_inst.ins.sync_info
    ow = list(si.on_wait) if si is not None else []
    if len(ow) > 1:
        si.on_wait = [ow[0]]
        for w in ow[1:]:
            d2 = self.nc.sync.drain()
            d2.ins.sync_info = SyncInfo(on_wait=[w], on_update=[])
    self.nc.all_engine_barrier()
    assert self.sems is not None
    popped = self.nc._tile_sem_poison_stack.pop()
    assert popped is self._sem_poison
    self.nc.clear_and_free_semaphores(list(self.sems.allocated().values()))
    self.nc.all_engine_barrier()


tile.TileContext._drain_and_barrier = _drain_and_barrier


def _split_multiwait_instructions(nc):
    """Walrus here allows only one sync-wait per instruction: hoist extra
    waits onto same-engine NOPs inserted immediately before."""
    ctr = 0
    for fn in nc.m.functions:
        for bb in fn.blocks:
            insts = bb.instructions
            out = []
            changed = False
            for ins in insts:
                si = getattr(ins, "sync_info", None)
                ow = list(si.on_wait) if si is not None else []
                if len(ow) > 1:
                    changed = True
                    for w in ow[:-1]:
                        ctr += 1
                        out.append(
                            mybir.InstNoOp(
                                name=f"mwsplit-{ctr}",
                                sync_info=SyncInfo(on_wait=[w], on_update=[]),
                                engine=ins.engine,
                                bass_nofuse=True,
                            )
                        )
                    si.on_wait = [ow[-1]]
                out.append(ins)
            if changed:
                bb.instructions = out
# ---------------------------------------------------------------------------

N_CORES = 8
B, S, IN, OUT, FP = 4, 2048, 4096, 4096, 256
INT = IN - FP                    # 3840 int features (compact order)
NT = (B * S) // N_CORES          # 1024 tokens per core
P = 128
KC = INT // P                    # 30 feature chunks of 128
CC = KC // 2                     # 15 DoubleRow chunks of 256
NOUT = 8                         # out-feature chunks
NSZ = OUT // NOUT                # 512
NGRP = 2                         # n-chunk groups (4 chunks each, 4 psum banks)
GS = NOUT // NGRP                # 4
HT = 2                           # token halves (512) for quantize layout
HSZ = NT // HT                   # 512
TOKT = NT // P                   # 8 token tiles of 128

f16 = mybir.dt.float16
f32 = mybir.dt.float32
f8 = mybir.dt.float8e4
i8 = mybir.dt.int8

_prog_cache = {}


def _build_program():
    nc = bass.Bass()
    xs = nc.declare_dram_parameter("xs", [NT, INT], f16, isOutput=False)
    xst = nc.declare_dram_parameter("xst", [INT, NT], f16, isOutput=False)
    fpxt = nc.declare_dram_parameter("fpxt", [FP, NT], f16, isOutput=False)
    w8n = nc.declare_dram_parameter("w8n", [NOUT, P, CC, 2, NSZ], f8, isOutput=False)
    wfp = nc.declare_dram_parameter("wfp", [FP, OUT], f16, isOutput=False)
    wfp2 = nc.declare_dram_parameter("wfp2", [2, OUT], f16, isOutput=False)
    wsrow = nc.declare_dram_parameter("wsrow", [OUT], f16, isOutput=False)
    out_d = nc.declare_dram_parameter("out", [NT, OUT], f16, isOutput=True)

    def bcast(ap, parts=P):
        # DRAM row -> all partitions: stride-0 partition dim, SWDGE DMA
        return bass.AP(
            tensor=ap.tensor, offset=ap.offset, ap=[[0, parts]] + list(ap.ap)
        )

    import contextlib
    with tile.TileContext(nc) as tc:
        with (
            tc.tile_pool(name="const", bufs=1) as cpool,
            tc.tile_pool(name="rt", bufs=1) as rtpool,
            tc.tile_pool(name="stat1", bufs=1) as s1pool,
            tc.tile_pool(name="psum", bufs=2, space="PSUM") as ppool,
            tc.tile_pool(name="dram", bufs=1, space="DRAM") as dpool,
        ):
            # ---- resident constants -------------------------------------
            wsB = cpool.tile([P, OUT], f16, tag="wsB")
            nc.gpsimd.dma_start(wsB[:], bcast(wsrow[:]))
            wfp0_s = cpool.tile([P, OUT], f16, tag="wfp0")
            nc.sync.dma_start(wfp0_s[:], wfp[0:P, :])
            wfp1_s = cpool.tile([P, OUT], f16, tag="wfp1")
            nc.sync.dma_start(wfp1_s[:], wfp[P:FP, :])
            wfp2_s = cpool.tile([2, OUT], f16, tag="wfp2")
            nc.sync.dma_start(wfp2_s[:], wfp2[:])

            # DRAM staging for per-token stats rows
            mn32_d = dpool.tile([NT], f32, tag="mn32")
            inv32_d = dpool.tile([NT], f32, tag="inv32")
            zrow_d = dpool.tile([NT], f16, tag="zrow")
            invrow_d = dpool.tile([NT], f16, tag="invrow")

            # ---- phase S: per-token min/max -> scale/inv/zero -----------
            scl = []  # per-tok-tile [128,1] f32 scale, kept for dequant
            sctx = contextlib.ExitStack()
            spool = sctx.enter_context(tc.tile_pool(name="stat", bufs=2))
            for t in range(TOKT):
                ts_ = slice(t * P, (t + 1) * P)
                xtile = spool.tile([P, INT], f16, tag="xtile")
                nc.sync.dma_start(xtile[:], xs[ts_, :])
                mn_t = s1pool.tile([P, 1], f32, tag=f"mn{t}")
                mx_t = s1pool.tile([P, 1], f32, tag=f"mx{t}")
                nc.vector.tensor_reduce(
                    mn_t[:], xtile[:], mybir.AxisListType.X, AluOpType.min
                )
                nc.vector.tensor_reduce(
                    mx_t[:], xtile[:], mybir.AxisListType.X, AluOpType.max
                )
                sc_t = s1pool.tile([P, 1], f32, tag=f"sc{t}")
                nc.vector.tensor_tensor(sc_t[:], mx_t[:], mn_t[:], AluOpType.subtract)
                nc.vector.tensor_scalar(
                    sc_t[:], sc_t[:], 1.0 / 15.0, 1e-8,
                    AluOpType.mult, AluOpType.max,
                )
                inv_t = s1pool.tile([P, 1], f32, tag=f"inv{t}")
                nc.vector.reciprocal(inv_t[:], sc_t[:])
                # Newton step: inv *= (2 - scale*inv)  -> f32-accurate 1/scale
                nwt = s1pool.tile([P, 1], f32, tag=f"nw{t}")
                nc.vector.tensor_tensor(nwt[:], sc_t[:], inv_t[:], AluOpType.mult)
                nc.vector.tensor_scalar(
                    nwt[:], nwt[:], -1.0, 2.0, AluOpType.mult, AluOpType.add
                )
                nc.vector.tensor_tensor(inv_t[:], inv_t[:], nwt[:], AluOpType.mult)
                # zrow = 8 + mn*inv  (f16), invrow = inv (f16)
                zr_t = s1pool.tile([P, 1], f16, tag=f"zr{t}")
                mi_t = s1pool.tile([P, 1], f32, tag=f"mi{t}")
                nc.vector.tensor_tensor(mi_t[:], mn_t[:], inv_t[:], AluOpType.mult)
                nc.vector.tensor_scalar(
                    zr_t[:], mi_t[:], 8.0, None, AluOpType.add
                )
                iv_t = s1pool.tile([P, 1], f16, tag=f"iv{t}")
                nc.scalar.copy(iv_t[:], inv_t[:])
                nc.sync.dma_start(mn32_d[ts_], mn_t[:])
                nc.sync.dma_start(inv32_d[ts_], inv_t[:])
                nc.sync.dma_start(zrow_d[ts_], zr_t[:])
                nc.sync.dma_start(invrow_d[ts_], iv_t[:])
                scl.append(sc_t)
            sctx.close()

            # ---- broadcasts + fp-outlier activations --------------------
            mnB = cpool.tile([P, NT], f32, tag="mnB")
            nc.gpsimd.dma_start(mnB[:], bcast(mn32_d[:]))
            invB = cpool.tile([P, NT], f32, tag="invB")
            nc.gpsimd.dma_start(invB[:], bcast(inv32_d[:]))

            # fp rows scaled by 1/scale per token:  fpt = fpx^T * inv
            fpt0 = cpool.tile([P, NT], f16, tag="fpt0")
            fpx0 = cpool.tile([P, NT], f16, tag="fpx0")
            nc.sync.dma_start(fpx0[:], fpxt[0:P, :])
            nc.vector.tensor_tensor(fpt0[:], fpx0[:], invB[:], AluOpType.mult)
            fpt1 = cpool.tile([P, NT], f16, tag="fpt1")
            fpx1 = cpool.tile([P, NT], f16, tag="fpx1")
            nc.sync.dma_start(fpx1[:], fpxt[P:FP, :])
            nc.vector.tensor_tensor(fpt1[:], fpx1[:], invB[:], AluOpType.mult)
            fpt2 = cpool.tile([2, NT], f16, tag="fpt2")
            nc.sync.dma_start(fpt2[0:1, :], zrow_d[:])
            nc.sync.dma_start(fpt2[1:2, :], invrow_d[:])

            # ---- phase Q: quantize into rt[c][h] [128, 2, 512] fp8 ------
            qctx = contextlib.ExitStack()
            qpool = qctx.enter_context(tc.tile_pool(name="qw", bufs=4))
            rt = [
                [
                    rtpool.tile(
                        [P, 2, HSZ], f8, name=f"rt{c}_{h}", tag=f"rt{c}_{h}"
                    )
                    for h in range(HT)
                ]
                for c in range(CC)
            ]
            for h in range(HT):
                hs = slice(h * HSZ, (h + 1) * HSZ)
                for k in range(KC):
                    xt = qpool.tile([P, HSZ], f16, tag="xt")
                    nc.sync.dma_start(xt[:], xst[k * P : (k + 1) * P, hs])
                    q = qpool.tile([P, HSZ], f32, tag="q")
                    nc.vector.tensor_tensor(
                        q[:], xt[:], mnB[:, hs], AluOpType.subtract
                    )
                    nc.vector.tensor_tensor(q[:], q[:], invB[:, hs], AluOpType.mult)
                    r8i = qpool.tile([P, HSZ], i8, tag="r8")
                    nc.scalar.copy(r8i[:], q[:])      # f32->i8: round-half-even
                    nc.scalar.activation(
                        rt[k // 2][h][:, k % 2, :], r8i[:],
                        mybir.ActivationFunctionType.Copy, bias=-8.0,
                    )  # i8->fp8 with the -8 zero shift, exact
            qctx.close()

            # ---- phase M: DoubleRow matmuls + dequant -------------------
            mctx = contextlib.ExitStack()
            wpool = mctx.enter_context(tc.tile_pool(name="wp", bufs=1))
            dqpool = mctx.enter_context(tc.tile_pool(name="dq", bufs=3))
            for g in range(NGRP):
                wns = []
                for s in range(GS):
                    n = g * GS + s
                    wn = wpool.tile([P, CC, 2, NSZ], f8, tag=f"w{s}")
                    nc.sync.dma_start(wn[:], w8n[n])
                    wns.append(wn)
                for t in range(TOKT):
                    h = t // (TOKT // HT)
                    tsl = slice((t % (TOKT // HT)) * P, (t % (TOKT // HT)) * P + P)
                    ts_ = slice(t * P, (t + 1) * P)
                    psums = [
                        ppool.tile([P, NSZ], f32, name=f"pi{s}", tag=f"pi{s}")
                        for s in range(GS)
                    ]
                    for c in range(CC):
                        lhs = rt[c][h][:, :, tsl]
                        for s in range(GS):
                            nc.tensor.matmul(
                                psums[s][:], lhs, wns[s][:, c],
                                start=(c == 0), stop=False,
                                perf_mode=mybir.MatmulPerfMode.DoubleRow,
                            )
                    for s in range(GS):
                        ns = slice((g * GS + s) * NSZ, (g * GS + s + 1) * NSZ)
                        nc.tensor.matmul(
                            psums[s][:], fpt0[:, ts_], wfp0_s[:, ns],
                            start=False, stop=False,
                        )
                        nc.tensor.matmul(
                            psums[s][:], fpt1[:, ts_], wfp1_s[:, ns],
                            start=False, stop=False,
                        )
                        nc.tensor.matmul(
                            psums[s][:], fpt2[:, ts_], wfp2_s[:, ns],
                            start=False, stop=True,
                        )
                    for s in range(GS):
                        ns = slice((g * GS + s) * NSZ, (g * GS + s + 1) * NSZ)
                        td = dqpool.tile([P, NSZ], f16, tag="td")
                        nc.scalar.activation(
                            td[:], psums[s][:], mybir.ActivationFunctionType.Copy,
                            scale=scl[t][:, 0:1],
                        )
                        outt = dqpool.tile([P, NSZ], f16, tag="outt")
                        nc.vector.tensor_tensor(
                            outt[:], td[:], wsB[:, ns], AluOpType.mult
                        )
                        nc.sync.dma_start(out_d[ts_, ns], outt[:])
            mctx.close()
    _split_multiwait_instructions(nc)
    return nc


def _get_program():
    if "nc" not in _prog_cache:
        _prog_cache["nc"] = _build_program()
    return _prog_cache["nc"]


def _prep_shared(int_weight, fp_weight, bias, weights_scales, reduced_w):
    """Host-side weight layouts (shared across cores)."""
    wint = np.asarray(int_weight).astype(np.float32)          # [OUT, INT]
    ws32 = np.asarray(weights_scales, dtype=np.float32).reshape(OUT, 1)
    # w8n[n, p, c, j, o'] = wint[n*NSZ+o', c*256 + j*128 + p]
    wT = np.ascontiguousarray(wint.T)                         # [INT, OUT]
    w8 = wT.reshape(CC, 2, P, NOUT, NSZ).transpose(3, 2, 0, 1, 4)
    w8n = np.ascontiguousarray(w8).astype(ml_dtypes.float8_e4m3)
    # fp weights scaled by 1/ws, transposed
    wfpT = (np.asarray(fp_weight, dtype=np.float32) / ws32).T  # [FP, OUT]
    wfp = np.ascontiguousarray(wfpT).astype(np.float16)
    # extra contraction rows: [rw/ws = sum_k wint (exact ints), bias/ws]
    row_rw = wint.sum(axis=1)                                  # [OUT]
    row_bias = np.asarray(bias, dtype=np.float32) / ws32[:, 0]
    wfp2 = np.stack([row_rw, row_bias]).astype(np.float16)
    wsrow = np.ascontiguousarray(
        np.asarray(weights_scales, dtype=np.float16).reshape(-1)
    )
    return w8n, wfp, wfp2, wsrow


def _make_in_maps(x, int_weight, fp_weight, bias, weights_scales, reduced_w,
                  int_indices, fp_indices):
    x2 = np.asarray(x, dtype=np.float16).reshape(-1, IN)
    ii = np.asarray(int_indices).astype(np.int64)
    fi = np.asarray(fp_indices).astype(np.int64)

    w8n, wfp, wfp2, wsrow = _prep_shared(
        int_weight, fp_weight, bias, weights_scales, reduced_w
    )
    xint = x2[:, ii]                                           # [N, INT]
    xfp = x2[:, fi]                                            # [N, FP]

    in_maps = []
    for c in range(N_CORES):
        sl = slice(c * NT, (c + 1) * NT)
        in_maps.append({
            "xs": np.ascontiguousarray(xint[sl]),
            "xst": np.ascontiguousarray(xint[sl].T),
            "fpxt": np.ascontiguousarray(xfp[sl].T),
            "w8n": w8n,
            "wfp": wfp,
            "wfp2": wfp2,
            "wsrow": wsrow,
        })
    return in_maps


def kernel(x, int_weight, fp_weight, bias, weights_scales, reduced_w,
           int_indices, fp_indices):
    in_maps = _make_in_maps(
        x, int_weight, fp_weight, bias, weights_scales, reduced_w,
        int_indices, fp_indices,
    )
    nc = _get_program()
    res = run_bass_kernel_spmd(nc, in_maps, list(range(N_CORES)))
    out = np.concatenate(
        [res.results[c]["out"] for c in range(N_CORES)], axis=0
    )
    return out.reshape(B, S, OUT).astype(np.float16)


# revision 17
# speedup vs baseline: 1.6931x; 1.2376x over previous
"""MixedQLinear (QUIK-style int4+fp16 outlier linear) on 8 TRN2 NeuronCores.

Sharding: token-parallel. x [4,2048,4096] -> 8192 tokens, 1024 per core;
weights replicated. Each core quantizes its tokens, runs the int4 GEMM in
fp8e4 DoubleRow mode (exact: products of ints in [-8,7] are exact in the
e6m3 PE datapath, accumulated in fp32 PSUM), and writes its [1024,4096]
output slice. Host concatenates.

Key algebra: with r = clip(round((x-mn)/scale),0,15) - 8,
  out = [ sum_k r*Wint + (fp_x/scale) @ (Wfp/ws)^T + (8+mn/scale)*(rw/ws)
          + (1/scale)*(bias/ws) ] * scale * ws
so the zero-point correction and bias ride as extra contraction rows of the
fp-outlier matmul (scaled by 1/scale per token), and dequant is one scaled
PSUM copy plus one multiply by ws.

Schedule notes (from trace analysis):
- Each DMA instruction drains one queue at ~25 GB/s; the HWDGE rings execute
  their DMA triggers in FIFO order. The sync ring therefore carries ONLY
  dependency-free loads (split into sub-512KB pieces, priority-ordered);
  dependent transfers ride the scalar ring.
- Per-token min/max comes from the transposed x tiles (the same tiles the
  quantizer reads): elementwise min/max trees split across Vector and
  GpSimd, a PE-transpose of the [128,512] accumulators into PSUM, then
  per-token-tile free-axis reduces. Broadcast rows for the quantizer are
  produced by ones[1,128] (x) row[1,512] matmuls into spare PSUM rotations.
- The int GEMM interleaves 4 psum banks per rt-chunk (stationary shared);
  measured matmul issue rate is ~216-222 ns per 512-col stream for both f16
  and DoubleRow. Phase M is emitted as (g0:t0-3), then half-1 stats, then
  (g0:t4-7), (g1:*) so the tensor stream never waits on half-1 stats.
"""

import numpy as np
import ml_dtypes
import concourse.bass as bass
import concourse.tile as tile
import concourse.mybir as mybir
from concourse.bass_utils import run_bass_kernel_spmd
from bass_rust import ScopedClock, SyncInfo
from concourse.alu_op_type import AluOpType

# ---------------------------------------------------------------------------
# Workaround: this toolchain's walrus accepts at most one sync-wait on a
# TPB_CTRL (Drain) instruction; Tile's tail drain attaches one wait per
# active DMA queue. Split it into a chain of single-wait drains.
def _drain_and_barrier(self, tick_clock, wait_clock):
    drain_inst = self.nc.sync.drain()
    wait_clock.add_sem_waits(
        drain_inst.ins, ScopedClock({None: tick_clock.global_clock})
    )
    si = drain_inst.ins.sync_info
    ow = list(si.on_wait) if si is not None else []
    if len(ow) > 1:
        si.on_wait = [ow[0]]
        for w in ow[1:]:
            d2 = self.nc.sync.drain()
            d2.ins.sync_info = SyncInfo(on_wait=[w], on_update=[])
    self.nc.all_engine_barrier()
    assert self.sems is not None
    popped = self.nc._tile_sem_poison_stack.pop()
    assert popped is self._sem_poison
    self.nc.clear_and_free_semaphores(list(self.sems.allocated().values()))
    self.nc.all_engine_barrier()


tile.TileContext._drain_and_barrier = _drain_and_barrier


def _split_multiwait_instructions(nc):
    """Walrus here allows only one sync-wait per instruction: hoist extra
    waits onto same-engine NOPs inserted immediately before."""
    ctr = 0
    for fn in nc.m.functions:
        for bb in fn.blocks:
            insts = bb.instructions
            out = []
            changed = False
            for ins in insts:
                si = getattr(ins, "sync_info", None)
                ow = list(si.on_wait) if si is not None else []
                if len(ow) > 1:
                    changed = True
                    for w in ow[:-1]:
                        ctr += 1
                        out.append(
                            mybir.InstNoOp(
                                name=f"mwsplit-{ctr}",
                                sync_info=SyncInfo(on_wait=[w], on_update=[]),
                                engine=ins.engine,
                                bass_nofuse=True,
                            )
                        )
                    si.on_wait = [ow[-1]]
                out.append(ins)
            if changed:
                bb.instructions = out
# ---------------------------------------------------------------------------

N_CORES = 8
B, S, IN, OUT, FP = 4, 2048, 4096, 4096, 256
INT = IN - FP                    # 3840 int features (compact order)
NT = (B * S) // N_CORES          # 1024 tokens per core
P = 128
KC = INT // P                    # 30 feature chunks of 128
CC = KC // 2                     # 15 DoubleRow chunks of 256
NOUT = 8                         # out-feature chunks
NSZ = OUT // NOUT                # 512
NGRP = 2                         # n-chunk groups (4 chunks each, 4 psum banks)
GS = NOUT // NGRP                # 4
HT = 2                           # token halves (512) for quantize layout
HSZ = NT // HT                   # 512
TOKT = NT // P                   # 8 token tiles of 128
TH = TOKT // HT                  # 4 token tiles per half

# Measured: the vector engine's f32->i8 output conversion is round-to-
# nearest-even, matching jnp.round exactly; no truncation compensation.
TRUNC_I8 = False

f16 = mybir.dt.float16
f32 = mybir.dt.float32
f8 = mybir.dt.float8e4
i8 = mybir.dt.int8

_prog_cache = {}


def _build_program():
    nc = bass.Bass()
    xst = nc.declare_dram_parameter("xst", [INT, NT], f16, isOutput=False)
    fpxt = nc.declare_dram_parameter("fpxt", [FP, NT], f16, isOutput=False)
    w8n = nc.declare_dram_parameter("w8n", [NOUT, P, CC, 2, NSZ], f8, isOutput=False)
    wfp = nc.declare_dram_parameter("wfp", [FP, OUT], f16, isOutput=False)
    wfp2 = nc.declare_dram_parameter("wfp2", [2, OUT], f16, isOutput=False)
    wsb_d = nc.declare_dram_parameter("wsb", [P, OUT], f16, isOutput=False)
    ident_d = nc.declare_dram_parameter("ident", [P, P], f16, isOutput=False)
    out_d = nc.declare_dram_parameter("out", [NT, OUT], f16, isOutput=True)

    with tile.TileContext(nc) as tc:
        with (
            tc.tile_pool(name="const", bufs=1) as cpool,
            tc.tile_pool(name="xt", bufs=1) as xtpool,
            tc.tile_pool(name="rt", bufs=1) as rtpool,
            tc.tile_pool(name="wp", bufs=1) as wpool,
            tc.tile_pool(name="st", bufs=2) as stpool,
            tc.tile_pool(name="s1", bufs=1) as s1pool,
            tc.tile_pool(name="q", bufs=3) as qpool,
            tc.tile_pool(name="dq", bufs=4) as dqpool,
            tc.tile_pool(name="psA", bufs=2, space="PSUM") as ppoolA,
            tc.tile_pool(name="psB", bufs=1, space="PSUM") as ppoolB,
            tc.tile_pool(name="tr", bufs=1, space="PSUM") as trpool,
            tc.tile_pool(name="dram", bufs=1, space="DRAM") as dpool,
        ):
            # ---- sync ring: all dependency-free loads, priority order ----
            def load_xt(h):
                hs = slice(h * HSZ, (h + 1) * HSZ)
                tiles = []
                for k in range(KC):
                    t_ = xtpool.tile([P, HSZ], f16, name=f"xt{k}", tag=f"xt{k}")
                    nc.sync.dma_start(t_[:], xst[k * P : (k + 1) * P, hs])
                    tiles.append(t_)
                return tiles

            QR0 = [(0, 4), (4, 8), (8, 12), (12, CC)]
            xts = [None, None]
            xts[0] = load_xt(0)
            wqs = [[None] * 4 for _ in range(GS)]
            for qi in (0, 1):
                c0, c1 = QR0[qi]
                for s in range(GS):
                    wt = wpool.tile(
                        [P, c1 - c0, 2, NSZ], f8,
                        name=f"w{s}q{qi}", tag=f"w{s}q{qi}",
                    )
                    nc.sync.dma_start(wt[:], w8n[s, :, c0:c1])
                    wqs[s][qi] = wt
            fpx0 = cpool.tile([P, NT], f16, tag="fpx0")
            nc.sync.dma_start(fpx0[:], fpxt[0:P, :])
            fpx1 = cpool.tile([P, NT], f16, tag="fpx1")
            nc.sync.dma_start(fpx1[:], fpxt[P:FP, :])
            ident = cpool.tile([P, P], f16, tag="ident")
            nc.sync.dma_start(ident[:], ident_d[:])
            wfp0_s = cpool.tile([P, OUT], f16, tag="wfp0")
            nc.sync.dma_start(wfp0_s[:, 0 : OUT // 2], wfp[0:P, 0 : OUT // 2])
            nc.sync.dma_start(wfp0_s[:, OUT // 2 :], wfp[0:P, OUT // 2 :])
            wfp1_s = cpool.tile([P, OUT], f16, tag="wfp1")
            nc.sync.dma_start(wfp1_s[:, 0 : OUT // 2], wfp[P:FP, 0 : OUT // 2])
            nc.sync.dma_start(wfp1_s[:, OUT // 2 :], wfp[P:FP, OUT // 2 :])
            wfp2_s = cpool.tile([2, OUT], f16, tag="wfp2")
            nc.sync.dma_start(wfp2_s[:], wfp2[:])
            wsB = cpool.tile([P, OUT], f16, tag="wsB")
            nc.sync.dma_start(wsB[:, 0 : OUT // 2], wsb_d[:, 0 : OUT // 2])
            nc.sync.dma_start(wsB[:, OUT // 2 :], wsb_d[:, OUT // 2 :])
            for qi in (2, 3):
                c0, c1 = QR0[qi]
                for s in range(GS):
                    wt = wpool.tile(
                        [P, c1 - c0, 2, NSZ], f8,
                        name=f"w{s}q{qi}", tag=f"w{s}q{qi}",
                    )
                    nc.sync.dma_start(wt[:], w8n[s, :, c0:c1])
                    wqs[s][qi] = wt
            xts[1] = load_xt(1)  # each chunk waits its h0 consumer, then flows

            fpt2 = cpool.tile([2, NT], f16, tag="fpt2")
            ones_t = cpool.tile([1, P], f32, tag="ones")
            nc.vector.memset(ones_t[:], 1.0)

            # DRAM staging rows (token-indexed)
            mnq32_d = dpool.tile([NT], f32, tag="mnq32")
            inv32_d = dpool.tile([NT], f32, tag="inv32")
            zrow_d = dpool.tile([NT], f16, tag="zrow")
            invrow_d = dpool.tile([NT], f16, tag="invrow")

            rt = [
                [
                    rtpool.tile(
                        [P, 2, HSZ], f8, name=f"rt{c}_{h}", tag=f"rt{c}_{h}"
                    )
                    for h in range(HT)
                ]
                for c in range(CC)
            ]
            scl = [None] * TOKT
            trees = {}

            # ---- per-half stats + quantize, split into emission slots ----
            # slot 0: tree part 1; slot 1: tree part 2; slot 2: transposes +
            # reduces + chains + stat stores; slot 3: rows + broadcasts +
            # fp scaling + quantize. Emitting the slots of half 1 between
            # phase-M(g0) token iterations keeps every engine stream
            # drained while half-0 GEMM work proceeds.
            def half_tree(h, part):
                xt = xts[h]
                if part == 0:
                    mna = stpool.tile([P, HSZ], f16, tag="mna")
                    nc.vector.tensor_tensor(
                        mna[:], xt[0][:], xt[1][:], AluOpType.min
                    )
                    mxa = stpool.tile([P, HSZ], f16, tag="mxa")
                    nc.vector.tensor_tensor(
                        mxa[:], xt[0][:], xt[1][:], AluOpType.max
                    )
                    trees[h] = (mna, mxa)
                    rng = range(2, KC // 2)
                else:
                    mna, mxa = trees[h]
                    rng = range(KC // 2, KC)
                for k in rng:
                    nc.vector.tensor_tensor(mna[:], mna[:], xt[k][:], AluOpType.min)
                    nc.vector.tensor_tensor(mxa[:], mxa[:], xt[k][:], AluOpType.max)
            def half_stats(h):
                mna, mxa = trees[h]
                # PE transpose both accumulators into one psum bank
                tr_t = trpool.tile([P, 2 * HSZ], f16, tag="tr")
                for b in range(TH):
                    bs = slice(b * P, (b + 1) * P)
                    nc.tensor.transpose(tr_t[:, bs], mna[:, bs], ident[:])
                for b in range(TH):
                    bs = slice(b * P, (b + 1) * P)
                    nc.tensor.transpose(
                        tr_t[:, HSZ + b * P : HSZ + (b + 1) * P],
                        mxa[:, bs], ident[:],
                    )
                # per-token-tile stat columns + scale chain
                for b in range(TH):
                    t = h * TH + b
                    ts_ = slice(t * P, (t + 1) * P)
                    mn_t = s1pool.tile([P, 1], f32, name=f"mn{t}", tag=f"mn{t}")
                    nc.vector.tensor_reduce(
                        mn_t[:], tr_t[:, b * P : (b + 1) * P],
                        mybir.AxisListType.X, AluOpType.min,
                    )
                    mx_t = s1pool.tile([P, 1], f32, name=f"mx{t}", tag=f"mx{t}")
                    nc.vector.tensor_reduce(
                        mx_t[:], tr_t[:, HSZ + b * P : HSZ + (b + 1) * P],
                        mybir.AxisListType.X, AluOpType.max,
                    )
                    sc_t = s1pool.tile([P, 1], f32, name=f"sc{t}", tag=f"sc{t}")
                    nc.vector.tensor_tensor(
                        sc_t[:], mx_t[:], mn_t[:], AluOpType.subtract
                    )
                    nc.vector.tensor_scalar(
                        sc_t[:], sc_t[:], 1.0 / 15.0, 1e-8,
                        AluOpType.mult, AluOpType.max,
                    )
                    inv_t = s1pool.tile([P, 1], f32, name=f"inv{t}", tag=f"inv{t}")
                    nc.vector.reciprocal(inv_t[:], sc_t[:])
                    nwt = s1pool.tile([P, 1], f32, name=f"nw{t}", tag=f"nw{t}")
                    nc.vector.tensor_tensor(nwt[:], sc_t[:], inv_t[:], AluOpType.mult)
                    nc.vector.tensor_scalar(
                        nwt[:], nwt[:], -1.0, 2.0, AluOpType.mult, AluOpType.add
                    )
                    nc.vector.tensor_tensor(inv_t[:], inv_t[:], nwt[:], AluOpType.mult)
                    mq_t = s1pool.tile([P, 1], f32, name=f"mq{t}", tag=f"mq{t}")
                    if TRUNC_I8:
                        nc.vector.tensor_scalar(
                            mq_t[:], sc_t[:], -0.5, None, AluOpType.mult
                        )
                        nc.vector.tensor_tensor(
                            mq_t[:], mq_t[:], mn_t[:], AluOpType.add
                        )
                    else:
                        nc.vector.tensor_scalar(
                            mq_t[:], mn_t[:], 1.0, None, AluOpType.mult
                        )
                    zr_t = s1pool.tile([P, 1], f32, name=f"zrf{t}", tag=f"zrf{t}")
                    nc.vector.tensor_tensor(zr_t[:], mn_t[:], inv_t[:], AluOpType.mult)
                    zr16 = s1pool.tile([P, 1], f16, name=f"zr{t}", tag=f"zr{t}")
                    nc.vector.tensor_scalar(
                        zr16[:], zr_t[:], 1.0, 8.0, AluOpType.mult, AluOpType.add
                    )
                    iv16 = s1pool.tile([P, 1], f16, name=f"iv{t}", tag=f"iv{t}")
                    nc.vector.tensor_scalar(
                        iv16[:], inv_t[:], 0.0, None, AluOpType.add
                    )
                    scl[t] = sc_t
                    # dependent stores ride the scalar ring
                    nc.scalar.dma_start(mnq32_d[ts_], mq_t[:])
                    nc.scalar.dma_start(inv32_d[ts_], inv_t[:])
                    nc.scalar.dma_start(zrow_d[ts_], zr16[:])
                    nc.scalar.dma_start(invrow_d[ts_], iv16[:])

            def half_quant(h):
                hs = slice(h * HSZ, (h + 1) * HSZ)
                xt = xts[h]
                # rows back from DRAM + broadcast via ones-matmul
                mq_row = stpool.tile([1, HSZ], f32, tag="mqrow")
                nc.scalar.dma_start(mq_row[:], mnq32_d[hs])
                iv_row = stpool.tile([1, HSZ], f32, tag="ivrow")
                nc.scalar.dma_start(iv_row[:], inv32_d[hs])
                nc.scalar.dma_start(fpt2[0:1, hs], zrow_d[hs])
                nc.scalar.dma_start(fpt2[1:2, hs], invrow_d[hs])
                bc0 = ppoolA.tile([P, NSZ], f32, name="pi0", tag="pi0")
                nc.tensor.matmul(bc0[:], ones_t[:], mq_row[:], start=True, stop=True)
                mnqB = stpool.tile([P, HSZ], f32, tag="mnqB")
                nc.vector.tensor_scalar(mnqB[:], bc0[:], 0.0, None, AluOpType.add)
                bc1 = ppoolA.tile([P, NSZ], f32, name="pi1", tag="pi1")
                nc.tensor.matmul(bc1[:], ones_t[:], iv_row[:], start=True, stop=True)
                invB = stpool.tile([P, HSZ], f32, tag="invB")
                nc.vector.tensor_scalar(invB[:], bc1[:], 0.0, None, AluOpType.add)
                # fp outlier rows scaled by inv (in place)
                nc.vector.tensor_tensor(
                    fpx0[:, hs], fpx0[:, hs], invB[:], AluOpType.mult
                )
                nc.vector.tensor_tensor(
                    fpx1[:, hs], fpx1[:, hs], invB[:], AluOpType.mult
                )
                # quantize chunks: sub+mult->i8 on vector, i8->f8 on scalar
                for k in range(KC):
                    q = qpool.tile([P, HSZ], f32, name="q", tag="q")
                    nc.vector.tensor_tensor(
                        q[:], xt[k][:], mnqB[:], AluOpType.subtract
                    )
                    r8i = qpool.tile([P, HSZ], i8, name="r8", tag="r8")
                    nc.vector.tensor_tensor(r8i[:], q[:], invB[:], AluOpType.mult)
                    nc.scalar.activation(
                        rt[k // 2][h][:, k % 2, :], r8i[:],
                        mybir.ActivationFunctionType.Copy, bias=-8.0,
                    )

            # ---- phase M: one token-tile iteration ----------------------
            # weights come as 4 quarter-tiles per n-chunk slot (c-ranges
            # 0:4, 4:8, 8:12, 12:15) so group-1 loads can start as soon as
            # group-0's early c-chunks retire.
            QR = [(0, 4), (4, 8), (8, 12), (12, CC)]

            def phase_m_t(g, t, wq):
                h = t // TH
                tsl = slice((t % TH) * P, (t % TH) * P + P)
                ts_ = slice(t * P, (t + 1) * P)
                psums = []
                for s in range(GS):
                    pool = ppoolA if s < 3 else ppoolB
                    psums.append(
                        pool.tile([P, NSZ], f32, name=f"pi{s}", tag=f"pi{s}")
                    )
                for c in range(CC):
                    qi = min(c // 4, 3)
                    lhs = rt[c][h][:, :, tsl]
                    for s in range(GS):
                        nc.tensor.matmul(
                            psums[s][:], lhs, wq[s][qi][:, c - QR[qi][0]],
                            start=(c == 0), stop=False,
                            perf_mode=mybir.MatmulPerfMode.DoubleRow,
                        )
                for s in range(GS):
                    ns = slice((g * GS + s) * NSZ, (g * GS + s + 1) * NSZ)
                    nc.tensor.matmul(
                        psums[s][:], fpx0[:, ts_], wfp0_s[:, ns],
                        start=False, stop=False,
                    )
                    nc.tensor.matmul(
                        psums[s][:], fpx1[:, ts_], wfp1_s[:, ns],
                        start=False, stop=False,
                    )
                    nc.tensor.matmul(
                        psums[s][:], fpt2[:, ts_], wfp2_s[:, ns],
                        start=False, stop=True,
                    )
                for s in range(GS):
                    ns = slice((g * GS + s) * NSZ, (g * GS + s + 1) * NSZ)
                    td = dqpool.tile([P, NSZ], f16, tag="td")
                    nc.scalar.activation(
                        td[:], psums[s][:], mybir.ActivationFunctionType.Copy,
                        scale=scl[t][:, 0:1],
                    )
                    outt = dqpool.tile([P, NSZ], f16, tag="outt")
                    nc.vector.tensor_tensor(
                        outt[:], td[:], wsB[:, ns], AluOpType.mult
                    )
                    nc.scalar.dma_start(out_d[ts_, ns], outt[:])

            # ---- schedule ----------------------------------------------
            half_tree(0, 0)
            half_tree(0, 1)
            half_stats(0)
            half_quant(0)
            phase_m_t(0, 0, wqs)
            half_tree(1, 0)
            phase_m_t(0, 1, wqs)
            half_tree(1, 1)
            phase_m_t(0, 2, wqs)
            half_stats(1)
            phase_m_t(0, 3, wqs)
            half_quant(1)
            for t in range(TH, TOKT):
                phase_m_t(0, t, wqs)
            wqs2 = []
            for s in range(GS):
                n = GS + s
                qs = []
                for qi, (c0, c1) in enumerate(QR):
                    wt = wpool.tile(
                        [P, c1 - c0, 2, NSZ], f8,
                        name=f"w{s}q{qi}", tag=f"w{s}q{qi}",
                    )
                    nc.sync.dma_start(wt[:], w8n[n, :, c0:c1])
                    qs.append(wt)
                wqs2.append(qs)
            for t in range(TOKT):
                phase_m_t(1, t, wqs2)
    _split_multiwait_instructions(nc)
    return nc


def _get_program():
    if "nc" not in _prog_cache:
        _prog_cache["nc"] = _build_program()
    return _prog_cache["nc"]


def _prep_shared(int_weight, fp_weight, bias, weights_scales, reduced_w):
    """Host-side weight layouts (shared across cores)."""
    wint = np.asarray(int_weight).astype(np.float32)          # [OUT, INT]
    ws32 = np.asarray(weights_scales, dtype=np.float32).reshape(OUT, 1)
    # w8n[n, p, c, j, o'] = wint[n*NSZ+o', c*256 + j*128 + p]
    wT = np.ascontiguousarray(wint.T)                         # [INT, OUT]
    w8 = wT.reshape(CC, 2, P, NOUT, NSZ).transpose(3, 2, 0, 1, 4)
    w8n = np.ascontiguousarray(w8).astype(ml_dtypes.float8_e4m3)
    # fp weights scaled by 1/ws, transposed
    wfpT = (np.asarray(fp_weight, dtype=np.float32) / ws32).T  # [FP, OUT]
    wfp = np.ascontiguousarray(wfpT).astype(np.float16)
    # extra contraction rows: [rw/ws = sum_k wint (exact ints), bias/ws]
    row_rw = wint.sum(axis=1)                                  # [OUT]
    row_bias = np.asarray(bias, dtype=np.float32) / ws32[:, 0]
    wfp2 = np.stack([row_rw, row_bias]).astype(np.float16)
    wsb = np.ascontiguousarray(
        np.broadcast_to(
            np.asarray(weights_scales, dtype=np.float16).reshape(1, OUT),
            (P, OUT),
        )
    )
    ident = np.eye(P, dtype=np.float16)
    return w8n, wfp, wfp2, wsb, ident


def _make_in_maps(x, int_weight, fp_weight, bias, weights_scales, reduced_w,
                  int_indices, fp_indices):
    x2 = np.asarray(x, dtype=np.float16).reshape(-1, IN)
    ii = np.asarray(int_indices).astype(np.int64)
    fi = np.asarray(fp_indices).astype(np.int64)

    w8n, wfp, wfp2, wsb, ident = _prep_shared(
        int_weight, fp_weight, bias, weights_scales, reduced_w
    )
    xint = x2[:, ii]                                           # [N, INT]
    xfp = x2[:, fi]                                            # [N, FP]

    in_maps = []
    for c in range(N_CORES):
        sl = slice(c * NT, (c + 1) * NT)
        in_maps.append({
            "xst": np.ascontiguousarray(xint[sl].T),
            "fpxt": np.ascontiguousarray(xfp[sl].T),
            "w8n": w8n,
            "wfp": wfp,
            "wfp2": wfp2,
            "wsb": wsb,
            "ident": ident,
        })
    return in_maps


def kernel(x, int_weight, fp_weight, bias, weights_scales, reduced_w,
           int_indices, fp_indices):
    in_maps = _make_in_maps(
        x, int_weight, fp_weight, bias, weights_scales, reduced_w,
        int_indices, fp_indices,
    )
    nc = _get_program()
    res = run_bass_kernel_spmd(nc, in_maps, list(range(N_CORES)))
    out = np.concatenate(
        [res.results[c]["out"] for c in range(N_CORES)], axis=0
    )
    return out.reshape(B, S, OUT).astype(np.float16)


# revision 26
# speedup vs baseline: 2.1514x; 1.2707x over previous
"""MixedQLinear (QUIK-style int4+fp16 outlier linear) on 8 TRN2 NeuronCores.

Sharding: token-parallel. x [4,2048,4096] -> 8192 tokens, 1024 per core;
weights replicated. Each core quantizes its tokens, runs the int4 GEMM in
fp8e4 DoubleRow mode (exact: products of ints in [-8,7] are exact in the
e6m3 PE datapath, accumulated in fp32 PSUM), and writes its [1024,4096]
output slice. Host concatenates.

Key algebra: with r = clip(round((x-mn)/scale),0,15) - 8,
  out = [ sum_k r*Wint + (fp_x/scale) @ (Wfp/ws)^T + (8+mn/scale)*(rw/ws)
          + (1/scale)*(bias/ws) ] * scale * ws
so the zero-point correction and bias ride as extra contraction rows of the
fp-outlier matmul (scaled by 1/scale per token), and dequant is one scaled
PSUM copy plus one multiply by ws.

Schedule notes (from trace analysis):
- Each DMA instruction drains one queue at ~25 GB/s; the HWDGE rings execute
  their DMA triggers in FIFO order. The sync ring therefore carries ONLY
  dependency-free loads (split into sub-512KB pieces, priority-ordered);
  dependent transfers ride the scalar ring.
- Per-token min/max comes from the transposed x tiles (the same tiles the
  quantizer reads): elementwise min/max trees split across Vector and
  GpSimd, a PE-transpose of the [128,512] accumulators into PSUM, then
  per-token-tile free-axis reduces. Broadcast rows for the quantizer are
  produced by ones[1,128] (x) row[1,512] matmuls into spare PSUM rotations.
- The int GEMM interleaves 4 psum banks per rt-chunk (stationary shared);
  measured matmul issue rate is ~216-222 ns per 512-col stream for both f16
  and DoubleRow. Phase M is emitted as (g0:t0-3), then half-1 stats, then
  (g0:t4-7), (g1:*) so the tensor stream never waits on half-1 stats.
"""

import numpy as np
import ml_dtypes
import concourse.bass as bass
import concourse.tile as tile
import concourse.mybir as mybir
from concourse.bass_utils import run_bass_kernel_spmd
from bass_rust import ScopedClock, SyncInfo
from concourse.alu_op_type import AluOpType

# ---------------------------------------------------------------------------
# Workaround: this toolchain's walrus accepts at most one sync-wait on a
# TPB_CTRL (Drain) instruction; Tile's tail drain attaches one wait per
# active DMA queue. Split it into a chain of single-wait drains.
def _drain_and_barrier(self, tick_clock, wait_clock):
    drain_inst = self.nc.sync.drain()
    wait_clock.add_sem_waits(
        drain_inst.ins, ScopedClock({None: tick_clock.global_clock})
    )
    si = drain_inst.ins.sync_info
    ow = list(si.on_wait) if si is not None else []
    if len(ow) > 1:
        si.on_wait = [ow[0]]
        for w in ow[1:]:
            d2 = self.nc.sync.drain()
            d2.ins.sync_info = SyncInfo(on_wait=[w], on_update=[])
    self.nc.all_engine_barrier()
    assert self.sems is not None
    popped = self.nc._tile_sem_poison_stack.pop()
    assert popped is self._sem_poison
    self.nc.clear_and_free_semaphores(list(self.sems.allocated().values()))
    self.nc.all_engine_barrier()


tile.TileContext._drain_and_barrier = _drain_and_barrier


def _split_multiwait_instructions(nc):
    """Walrus here allows only one sync-wait per instruction: hoist extra
    waits onto same-engine NOPs inserted immediately before."""
    ctr = 0
    for fn in nc.m.functions:
        for bb in fn.blocks:
            insts = bb.instructions
            out = []
            changed = False
            for ins in insts:
                si = getattr(ins, "sync_info", None)
                ow = list(si.on_wait) if si is not None else []
                if len(ow) > 1:
                    changed = True
                    for w in ow[:-1]:
                        ctr += 1
                        out.append(
                            mybir.InstNoOp(
                                name=f"mwsplit-{ctr}",
                                sync_info=SyncInfo(on_wait=[w], on_update=[]),
                                engine=ins.engine,
                                bass_nofuse=True,
                            )
                        )
                    si.on_wait = [ow[-1]]
                out.append(ins)
            if changed:
                bb.instructions = out
# ---------------------------------------------------------------------------

N_CORES = 8
B, S, IN, OUT, FP = 4, 2048, 4096, 4096, 256
INT = IN - FP                    # 3840 int features (compact order)
NT = (B * S) // N_CORES          # 1024 tokens per core
P = 128
KC = INT // P                    # 30 feature chunks of 128
CC = KC // 2                     # 15 DoubleRow chunks of 256
NOUT = 8                         # out-feature chunks
NSZ = OUT // NOUT                # 512
NGRP = 2                         # n-chunk groups (4 chunks each, 4 psum banks)
GS = NOUT // NGRP                # 4
HT = 2                           # token halves (512) for quantize layout
HSZ = NT // HT                   # 512
TOKT = NT // P                   # 8 token tiles of 128
TH = TOKT // HT                  # 4 token tiles per half

# Measured: the vector engine's f32->i8 output conversion is round-to-
# nearest-even, matching jnp.round exactly; no truncation compensation.
TRUNC_I8 = False

f16 = mybir.dt.float16
f32 = mybir.dt.float32
f8 = mybir.dt.float8e4
i8 = mybir.dt.int8

_prog_cache = {}


def _build_program():
    nc = bass.Bass()
    xst = nc.declare_dram_parameter("xst", [INT, NT], f16, isOutput=False)
    fpxt = nc.declare_dram_parameter("fpxt", [FP, NT], f16, isOutput=False)
    w8n = nc.declare_dram_parameter("w8n", [NOUT, P, CC, 2, NSZ], f8, isOutput=False)
    wfp = nc.declare_dram_parameter("wfp", [FP, OUT], f16, isOutput=False)
    wfp2 = nc.declare_dram_parameter("wfp2", [2, OUT], f16, isOutput=False)
    wsb_d = nc.declare_dram_parameter("wsb", [P, OUT], f16, isOutput=False)
    ident_d = nc.declare_dram_parameter("ident", [P, P], f16, isOutput=False)
    out_d = nc.declare_dram_parameter("out", [NT, OUT], f16, isOutput=True)

    with tile.TileContext(nc) as tc:
        with (
            tc.tile_pool(name="const", bufs=1) as cpool,
            tc.tile_pool(name="xt", bufs=1) as xtpool,
            tc.tile_pool(name="rt", bufs=1) as rtpool,
            tc.tile_pool(name="wp", bufs=1) as wpool,
            tc.tile_pool(name="st", bufs=2) as stpool,
            tc.tile_pool(name="s1", bufs=1) as s1pool,
            tc.tile_pool(name="q", bufs=3) as qpool,
            tc.tile_pool(name="dq", bufs=4) as dqpool,
            tc.tile_pool(name="psA", bufs=2, space="PSUM") as ppoolA,
            tc.tile_pool(name="psB", bufs=1, space="PSUM") as ppoolB,
            tc.tile_pool(name="tr", bufs=1, space="PSUM") as trpool,
            tc.tile_pool(name="dram", bufs=1, space="DRAM") as dpool,
        ):
            # ---- sync ring: all dependency-free loads, priority order ----
            def load_xt(h):
                hs = slice(h * HSZ, (h + 1) * HSZ)
                tiles = []
                for k in range(KC):
                    t_ = xtpool.tile([P, HSZ], f16, name=f"xt{k}", tag=f"xt{k}")
                    nc.sync.dma_start(t_[:], xst[k * P : (k + 1) * P, hs])
                    tiles.append(t_)
                return tiles

            QR0 = [(0, 4), (4, 8), (8, 12), (12, CC)]
            xts = [None, None]
            xts[0] = load_xt(0)
            wqs = [[None] * 4 for _ in range(GS)]
            for qi in (0, 1):
                c0, c1 = QR0[qi]
                for s in range(GS):
                    wt = wpool.tile(
                        [P, c1 - c0, 2, NSZ], f8,
                        name=f"w{s}q{qi}", tag=f"w{s}q{qi}",
                    )
                    nc.sync.dma_start(wt[:], w8n[s, :, c0:c1])
                    wqs[s][qi] = wt
            fpx0 = cpool.tile([P, NT], f16, tag="fpx0")
            nc.sync.dma_start(fpx0[:], fpxt[0:P, :])
            fpx1 = cpool.tile([P, NT], f16, tag="fpx1")
            nc.sync.dma_start(fpx1[:], fpxt[P:FP, :])
            ident = cpool.tile([P, P], f16, tag="ident")
            nc.sync.dma_start(ident[:], ident_d[:])
            wfp0_s = cpool.tile([P, OUT], f16, tag="wfp0")
            nc.sync.dma_start(wfp0_s[:, 0 : OUT // 2], wfp[0:P, 0 : OUT // 2])
            nc.sync.dma_start(wfp0_s[:, OUT // 2 :], wfp[0:P, OUT // 2 :])
            wfp1_s = cpool.tile([P, OUT], f16, tag="wfp1")
            nc.sync.dma_start(wfp1_s[:, 0 : OUT // 2], wfp[P:FP, 0 : OUT // 2])
            nc.sync.dma_start(wfp1_s[:, OUT // 2 :], wfp[P:FP, OUT // 2 :])
            wfp2_s = cpool.tile([2, OUT], f16, tag="wfp2")
            nc.sync.dma_start(wfp2_s[:], wfp2[:])
            wsB = cpool.tile([P, OUT], f16, tag="wsB")
            nc.sync.dma_start(wsB[:, 0 : OUT // 2], wsb_d[:, 0 : OUT // 2])
            nc.sync.dma_start(wsB[:, OUT // 2 :], wsb_d[:, OUT // 2 :])
            for qi in (2, 3):
                c0, c1 = QR0[qi]
                for s in range(GS):
                    wt = wpool.tile(
                        [P, c1 - c0, 2, NSZ], f8,
                        name=f"w{s}q{qi}", tag=f"w{s}q{qi}",
                    )
                    nc.sync.dma_start(wt[:], w8n[s, :, c0:c1])
                    wqs[s][qi] = wt
            xts[1] = load_xt(1)  # each chunk waits its h0 consumer, then flows

            fpt2 = cpool.tile([2, NT], f16, tag="fpt2")
            ones_t = cpool.tile([1, P], f32, tag="ones")
            nc.vector.memset(ones_t[:], 1.0)
            ident32 = cpool.tile([P, P], f32, tag="ident32")
            nc.scalar.copy(ident32[:], ident[:])

            rt = [
                [
                    rtpool.tile(
                        [P, 2, HSZ], f8, name=f"rt{c}_{h}", tag=f"rt{c}_{h}"
                    )
                    for h in range(HT)
                ]
                for c in range(CC)
            ]
            scl = [None] * TOKT
            trees = {}
            rows = {}
            trs = {}

            # ---- per-half stats + quantize, split into emission slots ----
            # slot 0: tree part 1; slot 1: tree part 2; slot 2: transposes +
            # reduces + chains + stat stores; slot 3: rows + broadcasts +
            # fp scaling + quantize. Emitting the slots of half 1 between
            # phase-M(g0) token iterations keeps every engine stream
            # drained while half-0 GEMM work proceeds.
            def half_tree(h, part):
                xt = xts[h]
                if part == 0:
                    mna = stpool.tile([P, HSZ], f16, tag="mna")
                    nc.vector.tensor_tensor(
                        mna[:], xt[0][:], xt[1][:], AluOpType.min
                    )
                    mxa = stpool.tile([P, HSZ], f16, tag="mxa")
                    nc.vector.tensor_tensor(
                        mxa[:], xt[0][:], xt[1][:], AluOpType.max
                    )
                    trees[h] = (mna, mxa)
                    rng = range(2, KC // 2)
                else:
                    mna, mxa = trees[h]
                    rng = range(KC // 2, KC)
                for k in rng:
                    nc.vector.tensor_tensor(mna[:], mna[:], xt[k][:], AluOpType.min)
                    nc.vector.tensor_tensor(mxa[:], mxa[:], xt[k][:], AluOpType.max)
            def half_stats(h):
                mna, mxa = trees[h]
                # psum row tiles for transposed mnq/inv stat columns (f32,
                # partition 0); two pi2 rotations, no extra PSUM bank
                rowtA = ppoolA.tile([P, NSZ], f32, name="pi2", tag="pi2")
                rowtB = ppoolA.tile([P, NSZ], f32, name="pi2", tag="pi2")
                rows[h] = (rowtA, rowtB)
                # PE transpose both accumulators into one psum bank
                tr_t = trpool.tile([P, 2 * HSZ], f16, tag="tr")
                trs[h] = tr_t
                for b in range(TH):
                    bs = slice(b * P, (b + 1) * P)
                    nc.tensor.transpose(tr_t[:, bs], mna[:, bs], ident[:])
                for b in range(TH):
                    bs = slice(b * P, (b + 1) * P)
                    nc.tensor.transpose(
                        tr_t[:, HSZ + b * P : HSZ + (b + 1) * P],
                        mxa[:, bs], ident[:],
                    )
                # per-token-tile stat columns + scale chain
                for b in range(TH):
                    t = h * TH + b
                    ts_ = slice(t * P, (t + 1) * P)
                    mn_t = s1pool.tile([P, 1], f32, name=f"mn{t}", tag=f"mn{t}")
                    nc.vector.tensor_reduce(
                        mn_t[:], tr_t[:, b * P : (b + 1) * P],
                        mybir.AxisListType.X, AluOpType.min,
                    )
                    mx_t = s1pool.tile([P, 1], f32, name=f"mx{t}", tag=f"mx{t}")
                    nc.vector.tensor_reduce(
                        mx_t[:], tr_t[:, HSZ + b * P : HSZ + (b + 1) * P],
                        mybir.AxisListType.X, AluOpType.max,
                    )
                    sc_t = s1pool.tile([P, 1], f32, name=f"sc{t}", tag=f"sc{t}")
                    nc.vector.tensor_tensor(
                        sc_t[:], mx_t[:], mn_t[:], AluOpType.subtract
                    )
                    nc.vector.tensor_scalar(
                        sc_t[:], sc_t[:], 1.0 / 15.0, 1e-8,
                        AluOpType.mult, AluOpType.max,
                    )
                    inv_t = s1pool.tile([P, 1], f32, name=f"inv{t}", tag=f"inv{t}")
                    nc.vector.reciprocal(inv_t[:], sc_t[:])
                    nwt = s1pool.tile([P, 1], f32, name=f"nw{t}", tag=f"nw{t}")
                    nc.vector.tensor_tensor(nwt[:], sc_t[:], inv_t[:], AluOpType.mult)
                    nc.vector.tensor_scalar(
                        nwt[:], nwt[:], -1.0, 2.0, AluOpType.mult, AluOpType.add
                    )
                    nc.vector.tensor_tensor(inv_t[:], inv_t[:], nwt[:], AluOpType.mult)
                    mq_t = s1pool.tile([P, 1], f32, name=f"mq{t}", tag=f"mq{t}")
                    if TRUNC_I8:
                        nc.vector.tensor_scalar(
                            mq_t[:], sc_t[:], -0.5, None, AluOpType.mult
                        )
                        nc.vector.tensor_tensor(
                            mq_t[:], mq_t[:], mn_t[:], AluOpType.add
                        )
                    else:
                        nc.vector.tensor_scalar(
                            mq_t[:], mn_t[:], 1.0, None, AluOpType.mult
                        )
                    zr_t = s1pool.tile([P, 1], f32, name=f"zrf{t}", tag=f"zrf{t}")
                    nc.vector.tensor_tensor(zr_t[:], mn_t[:], inv_t[:], AluOpType.mult)
                    zr16 = s1pool.tile([P, 1], f16, name=f"zr{t}", tag=f"zr{t}")
                    nc.vector.tensor_scalar(
                        zr16[:], zr_t[:], 1.0, 8.0, AluOpType.mult, AluOpType.add
                    )
                    iv16 = s1pool.tile([P, 1], f16, name=f"iv{t}", tag=f"iv{t}")
                    nc.vector.tensor_scalar(
                        iv16[:], inv_t[:], 0.0, None, AluOpType.add
                    )
                    scl[t] = sc_t
                    # PE-transpose the stat columns into row form (all at
                    # partition 0): mnq/inv (f32) into the pi2 rotations,
                    # zrow/invrow (f16) over the retired tree-transpose
                    # psum column ranges.
                    bs = slice(b * P, (b + 1) * P)
                    nc.tensor.transpose(rowtA[0:1, bs], mq_t[:], ident32[:])
                    nc.tensor.transpose(rowtB[0:1, bs], inv_t[:], ident32[:])
                    nc.tensor.transpose(tr_t[0:1, bs], zr16[:], ident[:])
                    nc.tensor.transpose(
                        tr_t[0:1, HSZ + b * P : HSZ + (b + 1) * P], iv16[:],
                        ident[:],
                    )

            def half_quant(h):
                hs = slice(h * HSZ, (h + 1) * HSZ)
                xt = xts[h]
                rowtA, rowtB = rows[h]
                tr_t = trs[h]
                # psum rows -> SBUF (partition-0 aligned copies)
                rwa = stpool.tile([1, HSZ], f32, tag="rwa")
                nc.vector.tensor_scalar(
                    rwa[:], rowtA[0:1, :], 0.0, None, AluOpType.add
                )
                rwb = stpool.tile([1, HSZ], f32, tag="rwb")
                nc.vector.tensor_scalar(
                    rwb[:], rowtB[0:1, :], 0.0, None, AluOpType.add
                )
                row16 = stpool.tile([1, 2 * HSZ], f16, tag="row16")
                nc.scalar.copy(row16[:], tr_t[0:1, :])
                nc.scalar.dma_start(fpt2[0:1, hs], row16[0:1, 0:HSZ])
                nc.scalar.dma_start(fpt2[1:2, hs], row16[0:1, HSZ:])
                bc0 = ppoolA.tile([P, NSZ], f32, name="pi0", tag="pi0")
                nc.tensor.matmul(
                    bc0[:], ones_t[:], rwa[:], start=True, stop=True
                )
                mnqB = stpool.tile([P, HSZ], f32, tag="mnqB")
                nc.vector.tensor_scalar(mnqB[:], bc0[:], 0.0, None, AluOpType.add)
                bc1 = ppoolA.tile([P, NSZ], f32, name="pi1", tag="pi1")
                nc.tensor.matmul(
                    bc1[:], ones_t[:], rwb[:], start=True, stop=True
                )
                invB = stpool.tile([P, HSZ], f32, tag="invB")
                nc.vector.tensor_scalar(invB[:], bc1[:], 0.0, None, AluOpType.add)
                # fp outlier rows scaled by inv (in place)
                nc.vector.tensor_tensor(
                    fpx0[:, hs], fpx0[:, hs], invB[:], AluOpType.mult
                )
                nc.vector.tensor_tensor(
                    fpx1[:, hs], fpx1[:, hs], invB[:], AluOpType.mult
                )
                # quantize chunks: sub+mult->i8 on vector, i8->f8 on scalar
                for k in range(KC):
                    q = qpool.tile([P, HSZ], f32, name="q", tag="q")
                    nc.vector.tensor_tensor(
                        q[:], xt[k][:], mnqB[:], AluOpType.subtract
                    )
                    r8i = qpool.tile([P, HSZ], i8, name="r8", tag="r8")
                    nc.vector.tensor_tensor(r8i[:], q[:], invB[:], AluOpType.mult)
                    nc.scalar.activation(
                        rt[k // 2][h][:, k % 2, :], r8i[:],
                        mybir.ActivationFunctionType.Copy, bias=-8.0,
                    )

            # ---- phase M: one token-tile iteration ----------------------
            # weights come as 4 quarter-tiles per n-chunk slot (c-ranges
            # 0:4, 4:8, 8:12, 12:15) so group-1 loads can start as soon as
            # group-0's early c-chunks retire.
            QR = [(0, 4), (4, 8), (8, 12), (12, CC)]

            def phase_m_t(g, t, wq):
                h = t // TH
                tsl = slice((t % TH) * P, (t % TH) * P + P)
                ts_ = slice(t * P, (t + 1) * P)
                psums = []
                for s in range(GS):
                    pool = ppoolA if s < 3 else ppoolB
                    psums.append(
                        pool.tile([P, NSZ], f32, name=f"pi{s}", tag=f"pi{s}")
                    )
                for c in range(CC):
                    qi = min(c // 4, 3)
                    lhs = rt[c][h][:, :, tsl]
                    for s in range(GS):
                        nc.tensor.matmul(
                            psums[s][:], lhs, wq[s][qi][:, c - QR[qi][0]],
                            start=(c == 0), stop=False,
                            perf_mode=mybir.MatmulPerfMode.DoubleRow,
                        )
                for s in range(GS):
                    ns = slice((g * GS + s) * NSZ, (g * GS + s + 1) * NSZ)
                    nc.tensor.matmul(
                        psums[s][:], fpx0[:, ts_], wfp0_s[:, ns],
                        start=False, stop=False,
                    )
                    nc.tensor.matmul(
                        psums[s][:], fpx1[:, ts_], wfp1_s[:, ns],
                        start=False, stop=False,
                    )
                    nc.tensor.matmul(
                        psums[s][:], fpt2[:, ts_], wfp2_s[:, ns],
                        start=False, stop=True,
                    )
                for s in range(GS):
                    ns = slice((g * GS + s) * NSZ, (g * GS + s + 1) * NSZ)
                    td = dqpool.tile([P, NSZ], f16, tag="td")
                    nc.scalar.activation(
                        td[:], psums[s][:], mybir.ActivationFunctionType.Copy,
                        scale=scl[t][:, 0:1],
                    )
                    outt = dqpool.tile([P, NSZ], f16, tag="outt")
                    nc.vector.tensor_tensor(
                        outt[:], td[:], wsB[:, ns], AluOpType.mult
                    )
                    nc.scalar.dma_start(out_d[ts_, ns], outt[:])

            # ---- schedule ----------------------------------------------
            half_tree(0, 0)
            half_tree(0, 1)
            half_stats(0)
            half_quant(0)
            phase_m_t(0, 0, wqs)
            half_tree(1, 0)
            phase_m_t(0, 1, wqs)
            half_tree(1, 1)
            phase_m_t(0, 2, wqs)
            half_stats(1)
            half_quant(1)
            phase_m_t(0, 3, wqs)
            for t in range(TH, TOKT):
                phase_m_t(0, t, wqs)
            wqs2 = []
            for s in range(GS):
                n = GS + s
                qs = []
                for qi, (c0, c1) in enumerate(QR):
                    wt = wpool.tile(
                        [P, c1 - c0, 2, NSZ], f8,
                        name=f"w{s}q{qi}", tag=f"w{s}q{qi}",
                    )
                    nc.sync.dma_start(wt[:], w8n[n, :, c0:c1])
                    qs.append(wt)
                wqs2.append(qs)
            for t in range(TOKT):
                phase_m_t(1, t, wqs2)
    _split_multiwait_instructions(nc)
    return nc


def _get_program():
    if "nc" not in _prog_cache:
        _prog_cache["nc"] = _build_program()
    return _prog_cache["nc"]


def _prep_shared(int_weight, fp_weight, bias, weights_scales, reduced_w):
    """Host-side weight layouts (shared across cores)."""
    wint = np.asarray(int_weight).astype(np.float32)          # [OUT, INT]
    ws32 = np.asarray(weights_scales, dtype=np.float32).reshape(OUT, 1)
    # w8n[n, p, c, j, o'] = wint[n*NSZ+o', c*256 + j*128 + p]
    wT = np.ascontiguousarray(wint.T)                         # [INT, OUT]
    w8 = wT.reshape(CC, 2, P, NOUT, NSZ).transpose(3, 2, 0, 1, 4)
    w8n = np.ascontiguousarray(w8).astype(ml_dtypes.float8_e4m3)
    # fp weights scaled by 1/ws, transposed
    wfpT = (np.asarray(fp_weight, dtype=np.float32) / ws32).T  # [FP, OUT]
    wfp = np.ascontiguousarray(wfpT).astype(np.float16)
    # extra contraction rows: [rw/ws = sum_k wint (exact ints), bias/ws]
    row_rw = wint.sum(axis=1)                                  # [OUT]
    row_bias = np.asarray(bias, dtype=np.float32) / ws32[:, 0]
    wfp2 = np.stack([row_rw, row_bias]).astype(np.float16)
    wsb = np.ascontiguousarray(
        np.broadcast_to(
            np.asarray(weights_scales, dtype=np.float16).reshape(1, OUT),
            (P, OUT),
        )
    )
    ident = np.eye(P, dtype=np.float16)
    return w8n, wfp, wfp2, wsb, ident


def _make_in_maps(x, int_weight, fp_weight, bias, weights_scales, reduced_w,
                  int_indices, fp_indices):
    x2 = np.asarray(x, dtype=np.float16).reshape(-1, IN)
    ii = np.asarray(int_indices).astype(np.int64)
    fi = np.asarray(fp_indices).astype(np.int64)

    w8n, wfp, wfp2, wsb, ident = _prep_shared(
        int_weight, fp_weight, bias, weights_scales, reduced_w
    )
    xint = x2[:, ii]                                           # [N, INT]
    xfp = x2[:, fi]                                            # [N, FP]

    in_maps = []
    for c in range(N_CORES):
        sl = slice(c * NT, (c + 1) * NT)
        in_maps.append({
            "xst": np.ascontiguousarray(xint[sl].T),
            "fpxt": np.ascontiguousarray(xfp[sl].T),
            "w8n": w8n,
            "wfp": wfp,
            "wfp2": wfp2,
            "wsb": wsb,
            "ident": ident,
        })
    return in_maps


def kernel(x, int_weight, fp_weight, bias, weights_scales, reduced_w,
           int_indices, fp_indices):
    in_maps = _make_in_maps(
        x, int_weight, fp_weight, bias, weights_scales, reduced_w,
        int_indices, fp_indices,
    )
    nc = _get_program()
    res = run_bass_kernel_spmd(nc, in_maps, list(range(N_CORES)))
    out = np.concatenate(
        [res.results[c]["out"] for c in range(N_CORES)], axis=0
    )
    return out.reshape(B, S, OUT).astype(np.float16)
